# revision 72
# baseline (speedup 1.0000x reference)
"""Distributed causal self-attention (RoPE) kernel for 8 TRN2 NeuronCores.

Reference semantics (b=2, s=2048, d=1024, 16 heads, hd=64, fp32):
    q/k/v = x @ W{q,k,v}.T ; q,k = rope(q,k) ; causal softmax(q k^T/sqrt(hd)) @ v ; @ Wo.T

Sharding: core c -> batch (c // 4), head-group (c % 4) [4 heads = 256 dims].
Tensor-parallel column split of Wq/Wk/Wv, row split of Wo; the row-parallel
partial outputs are summed on the host (the unshard for this decomposition).
No device collectives.

Compute dtype: bf16 matmul operands and rope tables, fp32 PSUM
accumulation.  The head-dim basis is permuted per head to
[even dims | odd dims] (dot-product invariant, applied consistently to q
and k) so RoPE's rotate-half partner swap is a clean 32-partition-block
swap done by SBUF->SBUF DMA; the rope elementwise chain runs all-bf16 on
DVE after a single PSUM evacuation cast.  Softmax: scores are tiny
(|s| < 4) so no max subtraction; exp on ScalarE; the denominator comes
from a ones-column appended to V (row 64 of the ctx^T matmul
accumulator, exact in fp32), its reciprocal broadcast across partitions
via a tiny SP DMA hop + gpsimd partition_broadcast.

Schedule (PE is the bottleneck engine; everything serves keeping it
busy): single PSUM pool scope for the whole kernel (spsum 4 banks +
cpsum 2 + qkpsum 2, no mid-kernel pool transitions).  The attention
inner loop is software-pipelined: scores(j+1) is emitted on the PE
queue BEFORE ctx(j), so the PE never sits behind a ctx matmul that
waits on exp(j).  Window epilogues first evacuate the ctx accumulator
to SBUF with one DVE copy (freeing the PSUM bank immediately); the
reciprocal/broadcast/divide chain runs off the critical path, with the
divide and odd-head pack hop deferred to the next window boundary.
Second-pair q/k projection chunks are interleaved one per window into
pair-0 attention (PE filler for the Act-bound inner loop), and the
output projection windows 0..2 are interleaved into pair-1 attention;
only window 3's output projection remains as the tail.  Rope final
adds are deferred a chunk so they never head-of-line-block the DVE
queue behind the partner-swap DMA.
"""

import numpy as np
import ml_dtypes

import concourse.bass as bass
import concourse.mybir as mybir
import concourse.tile as tile
from concourse import bacc
from concourse.bass_utils import run_bass_kernel_spmd

P = 128
B, S, D = 2, 2048, 1024
NH, HD = 16, 64
NCORES = 8
HG = 4                 # heads per core
C = HG * HD            # 256 projected dims per core
THETA = 10000.0
F32 = mybir.dt.float32
BF16 = mybir.dt.bfloat16
BF = ml_dtypes.bfloat16

AX = mybir.AluOpType


def head_perm():
    """Per-head dim permutation: [0,2,...,62, 1,3,...,63]."""
    return np.arange(HD).reshape(HD // 2, 2).T.reshape(-1)


def rope_tables(s=S):
    """cosF/sinF [P, s] fp32 for the T-layout permuted basis.

    Row r (within a 128-row tile covering two heads): freq f = r % 32.
    sinF here is the PRE-SWAP table T with T[q] = S(partner(q)) * sin,
    i.e. +sin on the x1 half (r % 64 < 32), -sin on the x2 half, so that
    after the partner-block swap of t2pre = ps * T the rotate-half term
    lands with the right sign (see build_kernel).
    """
    inv = 1.0 / (THETA ** (np.arange(0, HD, 2, dtype=np.float64) / HD))  # [32]
    pos = np.arange(s, dtype=np.float64)
    r = np.arange(P)
    ang = pos[None, :] * inv[r % 32][:, None]          # [128, s]
    cosf = np.cos(ang).astype(np.float32)
    sign = np.where((r % 64) < 32, 1.0, -1.0)[:, None]
    sinf = (np.sin(ang) * sign).astype(np.float32)
    return cosf, sinf


def build_kernel(s=S, dbg=False, repeat=1):
    """Build the per-core Bass graph (same SPMD graph for all 8 cores)."""
    KT = D // P            # k-tiles over the model dim (8)
    CT = C // P            # partition tiles over this core's 256 dims (2)
    TT = s // P            # token tiles (16)
    NW = s // 512          # 512-wide q windows
    NEG = -1.0e30

    nc = bacc.Bacc("TRN2", target_bir_lowering=False, debug=False)

    xT_d = nc.dram_tensor("xT", [D, s], BF16, kind="ExternalInput").ap()
    wqT_d = nc.dram_tensor("wqT", [D, C], BF16, kind="ExternalInput").ap()
    wkT_d = nc.dram_tensor("wkT", [D, C], BF16, kind="ExternalInput").ap()
    wvT_d = nc.dram_tensor("wvT", [D, C], BF16, kind="ExternalInput").ap()
    woT_d = nc.dram_tensor("woT", [C, D], BF16, kind="ExternalInput").ap()
    cosf_d = nc.dram_tensor("cosf", [P, s], BF16, kind="ExternalInput").ap()
    sinf_d = nc.dram_tensor("sinf", [P, s], BF16, kind="ExternalInput").ap()
    out_d = nc.dram_tensor("out", [s, D], BF16, kind="ExternalOutput").ap()

    with tile.TileContext(nc) as tc:
      with (
          tc.tile_pool(name="persist", bufs=1) as persist,
          tc.tile_pool(name="small", bufs=3) as small,
      ):
        # ---- persistent SBUF staging ----
        wqT = persist.tile([P, KT, C], BF16, tag="wqT")
        wkT = persist.tile([P, KT, C], BF16, tag="wkT")
        wvT = persist.tile([P, KT, C], BF16, tag="wvT")
        woT = persist.tile([P, CT, D], BF16, tag="woT")
        cosf = persist.tile([P, s], BF16, tag="cosf")
        sinf = persist.tile([P, s], BF16, tag="sinf")
        qT = persist.tile([P, CT, s], BF16, tag="qT")
        kT = persist.tile([P, CT, s], BF16, tag="kT")
        # v with a ones column per head: [.., h*65+64] == 1.0
        vsb = persist.tile([P, TT, HG * (HD + 1)], BF16, tag="v")
        ctx_pack = persist.tile([P, CT, s], BF16, tag="ctxp")
        ctx_odd = persist.tile([P, CT, s], BF16, tag="ctxo")
        bmask = persist.tile([P, P], BF16, tag="bmask")

        for rep in range(repeat):
            with tc.tile_pool(name=f"xpool{rep}", bufs=1) as xpool, \
                 tc.tile_pool(name=f"ropet{rep}", bufs=3) as ropet, \
                 tc.tile_pool(name=f"attn{rep}", bufs=8) as attnp, \
                 tc.tile_pool(name=f"ctxs{rep}", bufs=4) as ctxsp, \
                 tc.tile_pool(name=f"ostage{rep}", bufs=2) as ostage, \
                 tc.tile_pool(name=f"spsum{rep}", bufs=2, space="PSUM") as spsum, \
                 tc.tile_pool(name=f"cpsum{rep}", bufs=2, space="PSUM") as cpsum, \
                 tc.tile_pool(name=f"qkpsum{rep}", bufs=2, space="PSUM") as qkpsum:
                xT = xpool.tile([P, KT, s], BF16, tag="xT", name="xT")
                xv = xT_d.rearrange("(a p) s -> p a s", p=P)
                # loads ordered by first use: v projection starts on the
                # first token tile, weights/tables land just before their
                # consumers, wo (tail-only) goes last
                def ldx(lo, hi):
                    if hi > lo:
                        nc.sync.dma_start(xT[:, :, lo:hi], xv[:, :, lo:hi])
                ldx(0, P)
                nc.sync.dma_start(wvT[:], wvT_d.rearrange("(a p) c -> p a c", p=P))
                ldx(P, max(s // 4, P))
                nc.sync.dma_start(wqT[:], wqT_d.rearrange("(a p) c -> p a c", p=P))
                nc.sync.dma_start(wkT[:], wkT_d.rearrange("(a p) c -> p a c", p=P))
                ldx(max(s // 4, P), s // 2)
                nc.sync.dma_start(sinf[:, 0: s // 2], sinf_d[:, 0: s // 2])
                nc.sync.dma_start(cosf[:, 0: s // 2], cosf_d[:, 0: s // 2])
                ldx(s // 2, 3 * s // 4)
                nc.sync.dma_start(sinf[:, s // 2: s], sinf_d[:, s // 2: s])
                nc.sync.dma_start(cosf[:, s // 2: s], cosf_d[:, s // 2: s])
                ldx(3 * s // 4, s)
                nc.sync.dma_start(woT[:], woT_d.rearrange("(a p) d -> p a d", p=P))

                # 0/1 causal mask [k-row, q-col]: 1 where qcol >= krow.
                # Applied multiplicatively to at AFTER the exp (bf16 DVE
                # 2x) so the scores->exp chain has no mask hop in it.
                nc.gpsimd.memset(bmask[:], 1.0)
                nc.gpsimd.affine_select(
                    out=bmask[:], in_=bmask[:],
                    compare_op=AX.is_ge, fill=0.0,
                    base=0, pattern=[[1, P]], channel_multiplier=-1,
                )
                # only the per-head ones columns need initializing; the v
                # projection fills the rest.
                nc.gpsimd.memset(
                    vsb[:].rearrange("p t (h e) -> p t h e", e=HD + 1)[
                        :, :, :, HD: HD + 1], 1.0)

                # ---- v projection (xT stationary -> natural layout).
                # Prologue tiles evacuate on ScalarE (idle until the
                # first exp); tiles deferred into the attention filler
                # queue evacuate on gpsimd (ScalarE is exp-bound there).
                def v_tile_parts(t, evac_eng):
                    state = {}

                    def half(k0):
                        def part():
                            if k0 == 0:
                                state["pv"] = qkpsum.tile(
                                    [P, 512], F32, tag="qk", name=f"pv_{t}")
                            pv = state["pv"]
                            for kt in range(k0, k0 + KT // 2):
                                nc.tensor.matmul(
                                    pv[:, 0:C],
                                    lhsT=xT[:, kt, P * t: P * t + P],
                                    rhs=wvT[:, kt, :],
                                    start=(kt == 0), stop=(kt == KT - 1),
                                )
                            if k0 == 0:
                                return
                            dst = vsb[:, t, :].rearrange(
                                "p (h e) -> p h e", e=HD + 1)[:, :, 0:HD]
                            src = pv[:, 0:C].rearrange(
                                "p (h e) -> p h e", e=HD)
                            if evac_eng == "s":
                                nc.scalar.copy(dst, src)
                            else:
                                nc.vector.tensor_copy(dst, src)
                        return part

                    return [half(0), half(KT // 2)]

                def v_tile(t, evac_eng):
                    for it in v_tile_parts(t, evac_eng):
                        it()

                for t in range(TT // 2):
                    v_tile(t, "s")

                # ---- q/k projection chunk, rope chain with deferred add ----
                pending_rope = []

                def flush_rope(n=None):
                    k = len(pending_rope) if n is None else min(n, len(pending_rope))
                    for _ in range(k):
                        pending_rope.pop(0)()

                def qk_chunk_parts(wT, outT, m, ck, scalar_evac,
                                   swap_sync=False):
                    """Two emission items (so the attention loop can pump
                    half a chunk per inner step as PE filler)."""
                    fs = 512 * ck
                    state = {}

                    def mm_part(k0):
                        def part():
                            if k0 == 0:
                                state["ps"] = qkpsum.tile(
                                    [P, 512], F32, tag="qk",
                                    name=f"ps_{m}_{ck}")
                            ps = state["ps"]
                            for kt in range(k0, k0 + 2):
                                nc.tensor.matmul(
                                    ps[:],
                                    lhsT=wT[:, kt, P * m: P * m + P],
                                    rhs=xT[:, kt, fs: fs + 512],
                                    start=(kt == 0), stop=(kt == KT - 1),
                                )
                        return part

                    def part2():
                        ps = state["ps"]
                        # evacuate the projection to bf16 SBUF once, then
                        # run the rope elementwise ops all-16-bit (DVE 2x)
                        psb = ropet.tile([P, 512], BF16, tag="psb")
                        if scalar_evac:
                            nc.scalar.copy(psb[:], ps[:])
                        else:
                            nc.vector.tensor_copy(psb[:], ps[:])
                        # t2pre[r] = psb[r] * sinF[partner(r)]; partner
                        # swap happens SBUF->SBUF by DMA (DMA cannot
                        # cross partitions on compute engines)
                        t2pre = ropet.tile([P, 512], BF16, tag="t2pre")
                        nc.vector.tensor_tensor(
                            t2pre[:], psb[:], sinf[:, fs: fs + 512],
                            op=AX.mult)
                        t2 = ropet.tile([P, 512], BF16, tag="t2")
                        for blk in range(4):
                            src = (blk ^ 1) * 32
                            eng = nc.sync if (swap_sync or blk % 2 == 0) \
                                else nc.gpsimd
                            eng.dma_start(
                                t2[32 * blk: 32 * blk + 32, :],
                                t2pre[src: src + 32, :])
                        t1 = ropet.tile([P, 512], BF16, tag="t1")
                        nc.vector.tensor_tensor(
                            t1[:], psb[:], cosf[:, fs: fs + 512],
                            op=AX.mult)

                        # the final add waits on the swap DMA (~1.3us);
                        # defer it so it never blocks the DVE queue head
                        def add():
                            nc.vector.tensor_tensor(
                                outT[:, m, fs: fs + 512], t1[:], t2[:],
                                op=AX.add)
                        pending_rope.append(add)

                    return [mm_part(0), mm_part(2), mm_part(4),
                            mm_part(6), part2]

                # ---- pair-0 projections (prologue; ScalarE evacuations).
                # The last chunk pair is deferred into the attention
                # filler queue -- the prologue would otherwise outrun the
                # HBM input loads. ----
                NPRO = max(1, NW - 1)     # prologue chunk count
                for ck in range(NPRO):
                    for it in qk_chunk_parts(wqT, qT, 0, ck,
                                             scalar_evac=True):
                        it()
                    for it in qk_chunk_parts(wkT, kT, 0, ck,
                                             scalar_evac=True):
                        it()
                    if ck:
                        flush_rope(2)
                flush_rope()

                # ---- deferred window-division epilogues ----
                pending_div = []

                def flush_div():
                    while pending_div:
                        pending_div.pop(0)()

                def epilogue(hpair, w, cps, direct=False):
                    """Evacuate ctx accumulators to SBUF (frees PSUM fast),
                    start the reciprocal/broadcast chain, defer the divide
                    and odd-head pack hop.  direct=True (kernel tail):
                    minimum-latency variant, nothing deferred.

                    Per head: denominator at accumulator row 64, ctx dims
                    at rows 0:64.  HW partition_broadcast only reads
                    partition 0, so the denominator row hops down via a
                    tiny DMA, is reciprocal'd there, and broadcast."""
                    ws = 512 * w
                    parts = []
                    for h2 in range(2):
                        cp = cps[h2]
                        if direct:
                            # tail: skip the evacuation hop -- reciprocal
                            # the whole accumulator straight from PSUM
                            # (offset-0 AP; the custom op is only unsafe
                            # at nonzero PSUM partition offsets)
                            src = cp
                            rec = small.tile([65, 512], F32, tag="rec")
                            nc.vector.reciprocal_approx_fast(
                                out=rec[0:65, :], in_=cp[0:65, :])
                            recr = rec[64:65, :]
                            rec0 = small.tile([1, 512], F32, tag="rec0")
                            (nc.sync if h2 == 0 else nc.scalar).dma_start(
                                rec0[:], rec[64:65, :])
                            recr = rec0[0:1, :]
                        else:
                            src = ctxsp.tile([P, 512], F32, tag="cs",
                                             name=f"cs_{hpair}_{w}_{h2}")
                            nc.vector.tensor_copy(src[0:65, :], cp[0:65, :])
                            rec0 = small.tile([1, 512], F32, tag="rec0")
                            (nc.sync if h2 == 0 else nc.gpsimd).dma_start(
                                rec0[:], src[64:65, :])
                            recr = small.tile([1, 512], F32, tag="recr")
                            nc.vector.reciprocal_approx_fast(
                                out=recr[0:1, :], in_=rec0[0:1, :])
                            recr = recr[0:1, :]
                        bcast = small.tile([64, 512], F32, tag="bc")
                        nc.gpsimd.partition_broadcast(bcast[:], recr)
                        parts.append((src, bcast))

                    def run(hpair=hpair, ws=ws, parts=parts,
                            direct=direct):
                        for h2, (src, bcast) in enumerate(parts):
                            nc.vector.tensor_tensor(
                                (ctx_pack if h2 == 0 else ctx_odd)[
                                    0:64, hpair, ws: ws + 512],
                                src[0:64, :], bcast[0:64, :], op=AX.mult)
                        # pack the odd-head rows into partitions 64:128;
                        # split across queues so the DMAs run in parallel
                        # (DMA-capable queues: SP, gpsimd, Activation)
                        nq = 3 if direct else 2
                        engs = [nc.sync, nc.gpsimd, nc.scalar]
                        bnds = [0, 32, 48, 64][:nq] + [64]
                        for i in range(nq):
                            lo, hi = bnds[i], bnds[i + 1]
                            engs[i].dma_start(
                                ctx_pack[64 + lo: 64 + hi,
                                         hpair, ws: ws + 512],
                                ctx_odd[lo: hi, hpair, ws: ws + 512])
                    if direct:
                        run()
                    else:
                        pending_div.append(run)

                # ---- output projection of one 512-token window, as a
                # list of small emission items (one [128,512] po tile
                # each) the attention loop pumps as PE filler.  cts
                # selects the ctx c-tiles to accumulate; addin names an
                # ostage tag holding a partial result to add on top ----
                def outproj_items(w, evac, cts=(0, 1), addin=None,
                                  otag="ot", store=True, pools=None):
                    ot = ostage.tile([P, 4, D], BF16, tag=otag,
                                     name=f"{otag}_{w}")
                    items = []
                    for ti in range(4):
                        t = 4 * w + ti
                        for nk in range(2):
                            def item(t=t, ti=ti, nk=nk):
                                if pools is None:
                                    po = qkpsum.tile([P, 512], F32,
                                                     tag="qk",
                                                     name=f"po_{t}_{nk}")
                                else:
                                    pool, tg = pools[(2 * ti + nk)
                                                     % len(pools)]
                                    po = pool.tile([P, 512], F32, tag=tg,
                                                   name=f"po_{t}_{nk}")
                                for i, ct in enumerate(cts):
                                    nc.tensor.matmul(
                                        po[:],
                                        lhsT=ctx_pack[:, ct,
                                                      P * t: P * t + P],
                                        rhs=woT[:, ct,
                                                512 * nk: 512 * nk + 512],
                                        start=(i == 0),
                                        stop=(i == len(cts) - 1),
                                    )
                                dst = ot[:, ti, 512 * nk: 512 * nk + 512]
                                e = evac[(2 * ti + nk) % len(evac)]
                                eng = {"s": nc.scalar, "v": nc.vector,
                                       "g": nc.gpsimd}[e]
                                if addin is None:
                                    if e == "s":
                                        eng.copy(dst, po[:])
                                    else:
                                        eng.tensor_copy(dst, po[:])
                                else:
                                    eng.tensor_tensor(
                                        dst, po[:],
                                        addin[:, ti,
                                              512 * nk: 512 * nk + 512],
                                        op=AX.add)
                            items.append(item)
                    if store == "split":
                        # one store per token tile, overlapping the
                        # remaining evacuations (kernel tail)
                        for ti in range(4):
                            items.insert(
                                2 * ti + 2 + ti,
                                lambda ti=ti: nc.sync.dma_start(
                                    out_d.rearrange(
                                        "(a p) d -> p a d", p=P)[
                                        :, 4 * w + ti: 4 * w + ti + 1, :],
                                    ot[:, ti: ti + 1, :]))
                    elif store:
                        items.append(lambda: nc.sync.dma_start(
                            out_d.rearrange("(a p) d -> p a d", p=P)[
                                :, 4 * w: 4 * w + 4, :], ot[:]))
                    return items, ot

                # ---- filler queue: small PE work items pumped one per
                # inner attention step, absorbing the per-j gap between
                # the PE's work (~0.73us) and ScalarE's exp (~0.87us) ----
                import collections as _c
                fillers = _c.deque()

                def pump(n=1):
                    for _ in range(n):
                        if not fillers:
                            return
                        fillers.popleft()()

                # ---- software-pipelined attention for one head pair ----
                def attention(hpair, boundary, boundary_j=1,
                              direct_last=False):
                    """boundary(w) is called at (w, j==boundary_j) -- it
                    refills the filler queue.  Deferred division
                    multiplies flush at (w, j==2), giving the broadcast
                    chain a head start so they never block the DVE queue
                    head (phase B's boundary runs at j==3, after the
                    flush, because its outproj consumes the divided ctx
                    of window w-1)."""
                    ch = hpair
                    prevs = _c.deque()   # 2-deep: (cps, w, j, jmax, d, at)

                    def emit_ctx(pv):
                        cps, pw, j, jmax, d, at = pv
                        for h2 in range(2):
                            h = 2 * hpair + h2
                            nc.tensor.matmul(
                                cps[h2][0:65, d: 512],
                                lhsT=vsb[:, j,
                                         (HD + 1) * h: (HD + 1) * h + HD + 1],
                                rhs=at[:, 512 * h2 + d: 512 * h2 + 512],
                                start=(j == 0), stop=(j == jmax - 1),
                            )
                        if j == jmax - 1:
                            epilogue(hpair, pw, cps,
                                     direct=(direct_last and pw == NW - 1))

                    for w in range(NW):
                        ws = 512 * w
                        jmax = (ws + 512) // 128
                        cps = {h2: cpsum.tile([P, 512], F32, tag="c",
                                              name=f"cp_{hpair}_{w}_{h2}")
                               for h2 in range(2)}
                        for j in range(jmax):
                            if j == min(4, jmax - 1):
                                flush_div()
                            if j == min(boundary_j, jmax - 1):
                                boundary(w)
                            start = max(ws, 128 * j)
                            d = start - ws
                            sc = spsum.tile([P, 1024], F32, tag="s",
                                            name=f"sc_{hpair}_{w}_{j}")
                            for h2 in range(2):
                                rh = 64 * h2
                                nc.tensor.matmul(
                                    sc[:, 512 * h2 + d: 512 * h2 + 512],
                                    lhsT=kT[rh: rh + 64, ch,
                                            128 * j: 128 * j + 128],
                                    rhs=qT[rh: rh + 64, ch, start: ws + 512],
                                    start=True, stop=True,
                                )
                            # trailing ctx from TWO j's back: the PE
                            # reaches it only after two newer scores, so
                            # it clears the in-order wait queue without
                            # ever stalling on exp at the queue head
                            if len(prevs) == 2:
                                emit_ctx(prevs.popleft())
                            at = attnp.tile([P, 1024], BF16, tag="attn",
                                            name=f"at_{hpair}_{w}_{j}")
                            nc.scalar.activation(
                                at[:].rearrange(
                                    "p (b n) -> p b n", b=2)[:, :, d: 512],
                                sc[:].rearrange(
                                    "p (b n) -> p b n", b=2)[:, :, d: 512],
                                mybir.ActivationFunctionType.Exp,
                                bias=0.0, scale=0.125,
                            )
                            if 128 * j >= ws:
                                # diagonal block: exp ran unmasked; zero
                                # the upper triangle of at AFTERWARD (off
                                # the scores->exp chain -- it only gates
                                # ctx, which has slack).  Phase B's DVE
                                # queue is the congested one; use gpsimd
                                # there.
                                atv = at[:].rearrange(
                                    "p (b n) -> p b n", b=2)[:, :, d: d + P]
                                # (last window's masks stay on DVE so the
                                # Pool queue is clear for the tail's
                                # broadcast chain)
                                meng = nc.gpsimd if (
                                    hpair and w < NW - 1) else nc.vector
                                meng.tensor_tensor(
                                    atv, atv,
                                    bmask[:, None, :].broadcast_to(
                                        [P, 2, P]),
                                    op=AX.mult)
                            prevs.append((cps, w, j, jmax, d, at))
                            pump(2 if len(fillers) > 16 else 1)
                    # drain remaining fillers (PE cover for the last
                    # exps) -- phase A keeps a few so phase B's
                    # filler-less first window has PE cover -- then the
                    # trailing ctxs + final epilogue
                    # (full-size only: the kept items are the last pair-1
                    # chunk, not consumed until phase B's last window)
                    keep = 6 if (not direct_last and NW >= 4) else 0
                    pump(max(0, len(fillers) - keep))
                    while prevs:
                        emit_ctx(prevs.popleft())

                # ---- phase A: pair-0 attention; fillers are the
                # deferred last pair-0 chunk, the deferred v-projection
                # tiles, and pair-1 projection chunks ----
                if NW > NPRO:
                    fillers.extend(qk_chunk_parts(wqT, qT, 0, NW - 1,
                                                  scalar_evac=False,
                                                  swap_sync=True))
                    fillers.extend(qk_chunk_parts(wkT, kT, 0, NW - 1,
                                                  scalar_evac=False,
                                                  swap_sync=True))
                    fillers.append(lambda: flush_rope(2))
                for t in range(TT // 2, TT):
                    fillers.extend(v_tile_parts(t, "v"))

                def boundary_a(w):
                    fillers.extend(qk_chunk_parts(wqT, qT, 1, w,
                                                  scalar_evac=False,
                                                  swap_sync=True))
                    fillers.extend(qk_chunk_parts(wkT, kT, 1, w,
                                                  scalar_evac=False,
                                                  swap_sync=True))
                    fillers.append(lambda: flush_rope(2))

                attention(0, boundary_a)

                # ---- phase B: pair-1 attention; output projections of
                # early windows as filler.  Window NW-2's projection is
                # HELD BACK: it runs in the tail as the PE work covering
                # the last window's division chain. ----
                def boundary_b(w):
                    flush_rope()
                    if 1 <= w <= NW - 2:
                        items, _ = outproj_items(w - 1, evac=("v", "v", "s", "v"))
                        fillers.extend(items)

                attention(1, boundary_b, boundary_j=5, direct_last=True)

                # ---- tail: window NW-2's projection covers the direct
                # division chain of window NW-1, then NW-1's projection
                # (ScalarE is exp-free by now and takes the evacuations)
                flush_div()
                if NW >= 2:
                    items, _ = outproj_items(
                        NW - 2, evac=("v", "s"),
                        pools=((qkpsum, "qk"), (spsum, "s")))
                    for it in items:
                        it()
                items, _ = outproj_items(
                    NW - 1, evac=("s", "v"),
                    pools=((qkpsum, "qk"), (spsum, "s"), (cpsum, "c")),
                    store="split")
                for it in items:
                    it()

    nc.compile()
    return nc


def make_in_maps(x, Wq, Wk, Wv, Wo, s=S):
    """Host-side shard prep: per-core input dict."""
    perm = head_perm()
    cosf, sinf = rope_tables(s)
    in_maps = []
    for c in range(NCORES):
        bi, hg = c // HG, c % HG
        heads = np.arange(HG * hg, HG * hg + HG)
        pcols = np.concatenate([h * HD + perm for h in heads])   # permuted q/k cols
        vcols = np.concatenate([h * HD + np.arange(HD) for h in heads])
        in_maps.append({
            "xT": np.ascontiguousarray(x[bi].T).astype(BF),
            "wqT": np.ascontiguousarray(Wq[pcols, :].T).astype(BF),
            "wkT": np.ascontiguousarray(Wk[pcols, :].T).astype(BF),
            "wvT": np.ascontiguousarray(Wv[vcols, :].T).astype(BF),
            "woT": np.ascontiguousarray(Wo[:, vcols].T).astype(BF),
            "cosf": cosf.astype(BF),
            "sinf": sinf.astype(BF),
        })
    return in_maps


_CACHE = {}


def _compiled(s=S):
    if s not in _CACHE:
        _CACHE[s] = build_kernel(s)
    return _CACHE[s]


def kernel(x, Wq, Wk, Wv, Wo, trace=False):
    x = np.asarray(x, dtype=np.float32)
    in_maps = make_in_maps(x, np.asarray(Wq), np.asarray(Wk),
                           np.asarray(Wv), np.asarray(Wo))
    nc = _compiled()
    res = run_bass_kernel_spmd(nc, in_maps, core_ids=list(range(NCORES)),
                               trace=trace)
    out = np.zeros((B, S, D), dtype=np.float32)
    for c in range(NCORES):
        out[c // HG] += res.results[c]["out"].astype(np.float32)
    if trace:
        return out, res
    return out


# revision 74
# speedup vs baseline: 438.9236x; 438.9236x over previous
"""Distributed causal self-attention (RoPE) kernel for 8 TRN2 NeuronCores.

Reference semantics (b=2, s=2048, d=1024, 16 heads, hd=64, fp32):
    q/k/v = x @ W{q,k,v}.T ; q,k = rope(q,k) ; causal softmax(q k^T/sqrt(hd)) @ v ; @ Wo.T

Sharding: core c -> batch (c // 4), head-group (c % 4) [4 heads = 256 dims].
Tensor-parallel column split of Wq/Wk/Wv, row split of Wo; the row-parallel
partial outputs are summed on the host (the unshard for this decomposition).
No device collectives.

Compute dtype: bf16 matmul operands and rope tables, fp32 PSUM
accumulation.  The head-dim basis is permuted per head to
[even dims | odd dims] (dot-product invariant, applied consistently to q
and k) so RoPE's rotate-half partner swap is a clean 32-partition-block
swap done by SBUF->SBUF DMA; the rope elementwise chain runs all-bf16 on
DVE after a single PSUM evacuation cast.  Softmax: scores are tiny
(|s| < 4) so no max subtraction; exp on ScalarE; the denominator comes
from a ones-column appended to V (row 64 of the ctx^T matmul
accumulator, exact in fp32), its reciprocal broadcast across partitions
via a tiny SP DMA hop + gpsimd partition_broadcast.

Schedule (PE is the bottleneck engine -- ~113us of matmul rows at 1
row/cycle bf16 -- everything else serves keeping it busy): single PSUM
pool scope for the whole kernel (spsum 4 banks + cpsum 2 + qkpsum 2,
no mid-kernel pool transitions).  The attention inner loop is
software-pipelined two deep: ctx(j) is emitted on the PE queue after
scores(j+2), so the in-order PE wait queue never stalls on exp; the
causal mask is applied multiplicatively to at AFTER the exp (bf16 DVE
2x / gpsimd) so the scores->exp chain has no cross-engine hop in it.
All non-attention PE work (the deferred half of the v projection, the
last pair-0 q/k chunk, all pair-1 q/k chunks, early output-projection
windows) is queued as small filler items pumped 1-2 per inner step,
absorbing the per-step gap between PE work (~0.73us) and ScalarE's exp
(~0.87us).  Window epilogues evacuate the ctx accumulator to SBUF with
one DVE copy (freeing the PSUM bank immediately); the denominator-row
hop / reciprocal / partition-broadcast chain runs off the critical
path with the divide and queue-split odd-head pack hop deferred several
inner steps.  Rope final adds are deferred so they never
head-of-line-block the DVE queue behind the partner-swap DMA.  The
tail runs window NW-2's output projection as PE cover for the last
window's direct division chain, spreads the last po tiles over three
PSUM pools, and splits the final store per token tile.
"""

import numpy as np
import ml_dtypes

import concourse.bass as bass
import concourse.mybir as mybir
import concourse.tile as tile
from concourse import bacc
from concourse.bass_utils import run_bass_kernel_spmd

P = 128
B, S, D = 2, 2048, 1024
NH, HD = 16, 64
NCORES = 8
HG = 4                 # heads per core
C = HG * HD            # 256 projected dims per core
THETA = 10000.0
F32 = mybir.dt.float32
BF16 = mybir.dt.bfloat16
BF = ml_dtypes.bfloat16

AX = mybir.AluOpType


def head_perm():
    """Per-head dim permutation: [0,2,...,62, 1,3,...,63]."""
    return np.arange(HD).reshape(HD // 2, 2).T.reshape(-1)


def rope_tables(s=S):
    """cosF/sinF [P, s] fp32 for the T-layout permuted basis.

    Row r (within a 128-row tile covering two heads): freq f = r % 32.
    sinF here is the PRE-SWAP table T with T[q] = S(partner(q)) * sin,
    i.e. +sin on the x1 half (r % 64 < 32), -sin on the x2 half, so that
    after the partner-block swap of t2pre = ps * T the rotate-half term
    lands with the right sign (see build_kernel).
    """
    inv = 1.0 / (THETA ** (np.arange(0, HD, 2, dtype=np.float64) / HD))  # [32]
    pos = np.arange(s, dtype=np.float64)
    r = np.arange(P)
    ang = pos[None, :] * inv[r % 32][:, None]          # [128, s]
    cosf = np.cos(ang).astype(np.float32)
    sign = np.where((r % 64) < 32, 1.0, -1.0)[:, None]
    sinf = (np.sin(ang) * sign).astype(np.float32)
    return cosf, sinf


def build_kernel(s=S, dbg=False, repeat=1):
    """Build the per-core Bass graph (same SPMD graph for all 8 cores)."""
    KT = D // P            # k-tiles over the model dim (8)
    CT = C // P            # partition tiles over this core's 256 dims (2)
    TT = s // P            # token tiles (16)
    NW = s // 512          # 512-wide q windows
    NEG = -1.0e30

    nc = bacc.Bacc("TRN2", target_bir_lowering=False, debug=False)

    xT_d = nc.dram_tensor("xT", [D, s], BF16, kind="ExternalInput").ap()
    wqT_d = nc.dram_tensor("wqT", [D, C], BF16, kind="ExternalInput").ap()
    wkT_d = nc.dram_tensor("wkT", [D, C], BF16, kind="ExternalInput").ap()
    wvT_d = nc.dram_tensor("wvT", [D, C], BF16, kind="ExternalInput").ap()
    woT_d = nc.dram_tensor("woT", [C, D], BF16, kind="ExternalInput").ap()
    cosf_d = nc.dram_tensor("cosf", [P, s], BF16, kind="ExternalInput").ap()
    sinf_d = nc.dram_tensor("sinf", [P, s], BF16, kind="ExternalInput").ap()
    out_d = nc.dram_tensor("out", [s, D], BF16, kind="ExternalOutput").ap()

    with tile.TileContext(nc) as tc:
      with (
          tc.tile_pool(name="persist", bufs=1) as persist,
          tc.tile_pool(name="small", bufs=3) as small,
      ):
        # ---- persistent SBUF staging ----
        wqT = persist.tile([P, KT, C], BF16, tag="wqT")
        wkT = persist.tile([P, KT, C], BF16, tag="wkT")
        wvT = persist.tile([P, KT, C], BF16, tag="wvT")
        woT = persist.tile([P, CT, D], BF16, tag="woT")
        cosf = persist.tile([P, s], BF16, tag="cosf")
        sinf = persist.tile([P, s], BF16, tag="sinf")
        qT = persist.tile([P, CT, s], BF16, tag="qT")
        kT = persist.tile([P, CT, s], BF16, tag="kT")
        # v with a ones column per head: [.., h*65+64] == 1.0
        vsb = persist.tile([P, TT, HG * (HD + 1)], BF16, tag="v")
        ctx_pack = persist.tile([P, CT, s], BF16, tag="ctxp")
        ctx_odd = persist.tile([P, CT, s], BF16, tag="ctxo")
        bmask = persist.tile([P, P], BF16, tag="bmask")

        for rep in range(repeat):
            with tc.tile_pool(name=f"xpool{rep}", bufs=1) as xpool, \
                 tc.tile_pool(name=f"ropet{rep}", bufs=3) as ropet, \
                 tc.tile_pool(name=f"attn{rep}", bufs=8) as attnp, \
                 tc.tile_pool(name=f"ctxs{rep}", bufs=4) as ctxsp, \
                 tc.tile_pool(name=f"ostage{rep}", bufs=2) as ostage, \
                 tc.tile_pool(name=f"spsum{rep}", bufs=2, space="PSUM") as spsum, \
                 tc.tile_pool(name=f"cpsum{rep}", bufs=2, space="PSUM") as cpsum, \
                 tc.tile_pool(name=f"qkpsum{rep}", bufs=2, space="PSUM") as qkpsum:
                xT = xpool.tile([P, KT, s], BF16, tag="xT", name="xT")
                xv = xT_d.rearrange("(a p) s -> p a s", p=P)
                # loads ordered by first use: v projection starts on the
                # first token tile, weights/tables land just before their
                # consumers, wo (tail-only) goes last
                def ldx(lo, hi):
                    if hi > lo:
                        nc.sync.dma_start(xT[:, :, lo:hi], xv[:, :, lo:hi])
                ldx(0, P)
                nc.sync.dma_start(wvT[:], wvT_d.rearrange("(a p) c -> p a c", p=P))
                ldx(P, max(s // 4, P))
                nc.sync.dma_start(wqT[:], wqT_d.rearrange("(a p) c -> p a c", p=P))
                nc.sync.dma_start(wkT[:], wkT_d.rearrange("(a p) c -> p a c", p=P))
                ldx(max(s // 4, P), s // 2)
                nc.sync.dma_start(sinf[:, 0: s // 2], sinf_d[:, 0: s // 2])
                nc.sync.dma_start(cosf[:, 0: s // 2], cosf_d[:, 0: s // 2])
                ldx(s // 2, 3 * s // 4)
                nc.sync.dma_start(sinf[:, s // 2: s], sinf_d[:, s // 2: s])
                nc.sync.dma_start(cosf[:, s // 2: s], cosf_d[:, s // 2: s])
                ldx(3 * s // 4, s)
                nc.sync.dma_start(woT[:], woT_d.rearrange("(a p) d -> p a d", p=P))

                # 0/1 causal mask [k-row, q-col]: 1 where qcol >= krow.
                # Applied multiplicatively to at AFTER the exp (bf16 DVE
                # 2x) so the scores->exp chain has no mask hop in it.
                nc.gpsimd.memset(bmask[:], 1.0)
                nc.gpsimd.affine_select(
                    out=bmask[:], in_=bmask[:],
                    compare_op=AX.is_ge, fill=0.0,
                    base=0, pattern=[[1, P]], channel_multiplier=-1,
                )
                # only the per-head ones columns need initializing; the v
                # projection fills the rest.
                nc.gpsimd.memset(
                    vsb[:].rearrange("p t (h e) -> p t h e", e=HD + 1)[
                        :, :, :, HD: HD + 1], 1.0)

                # ---- v projection (xT stationary -> natural layout).
                # Prologue tiles evacuate on ScalarE (idle until the
                # first exp); tiles deferred into the attention filler
                # queue evacuate on gpsimd (ScalarE is exp-bound there).
                def v_tile_parts(t, evac_eng):
                    state = {}

                    def half(k0):
                        def part():
                            if k0 == 0:
                                state["pv"] = qkpsum.tile(
                                    [P, 512], F32, tag="qk", name=f"pv_{t}")
                            pv = state["pv"]
                            for kt in range(k0, k0 + KT // 2):
                                nc.tensor.matmul(
                                    pv[:, 0:C],
                                    lhsT=xT[:, kt, P * t: P * t + P],
                                    rhs=wvT[:, kt, :],
                                    start=(kt == 0), stop=(kt == KT - 1),
                                )
                            if k0 == 0:
                                return
                            dst = vsb[:, t, :].rearrange(
                                "p (h e) -> p h e", e=HD + 1)[:, :, 0:HD]
                            src = pv[:, 0:C].rearrange(
                                "p (h e) -> p h e", e=HD)
                            if evac_eng == "s":
                                nc.scalar.copy(dst, src)
                            else:
                                nc.vector.tensor_copy(dst, src)
                        return part

                    return [half(0), half(KT // 2)]

                def v_tile(t, evac_eng):
                    for it in v_tile_parts(t, evac_eng):
                        it()

                for t in range(TT // 2):
                    v_tile(t, "s")

                # ---- q/k projection chunk, rope chain with deferred add ----
                pending_rope = []

                def flush_rope(n=None):
                    k = len(pending_rope) if n is None else min(n, len(pending_rope))
                    for _ in range(k):
                        pending_rope.pop(0)()

                def qk_chunk_parts(wT, outT, m, ck, scalar_evac,
                                   swap_sync=False):
                    """Two emission items (so the attention loop can pump
                    half a chunk per inner step as PE filler)."""
                    fs = 512 * ck
                    state = {}

                    def mm_part(k0):
                        def part():
                            if k0 == 0:
                                state["ps"] = qkpsum.tile(
                                    [P, 512], F32, tag="qk",
                                    name=f"ps_{m}_{ck}")
                            ps = state["ps"]
                            for kt in range(k0, k0 + 2):
                                nc.tensor.matmul(
                                    ps[:],
                                    lhsT=wT[:, kt, P * m: P * m + P],
                                    rhs=xT[:, kt, fs: fs + 512],
                                    start=(kt == 0), stop=(kt == KT - 1),
                                )
                        return part

                    def part2():
                        ps = state["ps"]
                        # evacuate the projection to bf16 SBUF once, then
                        # run the rope elementwise ops all-16-bit (DVE 2x)
                        psb = ropet.tile([P, 512], BF16, tag="psb")
                        if scalar_evac:
                            nc.scalar.copy(psb[:], ps[:])
                        else:
                            nc.vector.tensor_copy(psb[:], ps[:])
                        # t2pre[r] = psb[r] * sinF[partner(r)]; partner
                        # swap happens SBUF->SBUF by DMA (DMA cannot
                        # cross partitions on compute engines)
                        t2pre = ropet.tile([P, 512], BF16, tag="t2pre")
                        nc.vector.tensor_tensor(
                            t2pre[:], psb[:], sinf[:, fs: fs + 512],
                            op=AX.mult)
                        t2 = ropet.tile([P, 512], BF16, tag="t2")
                        for blk in range(4):
                            src = (blk ^ 1) * 32
                            eng = nc.sync if (swap_sync or blk % 2 == 0) \
                                else nc.gpsimd
                            eng.dma_start(
                                t2[32 * blk: 32 * blk + 32, :],
                                t2pre[src: src + 32, :])
                        t1 = ropet.tile([P, 512], BF16, tag="t1")
                        nc.vector.tensor_tensor(
                            t1[:], psb[:], cosf[:, fs: fs + 512],
                            op=AX.mult)

                        # the final add waits on the swap DMA (~1.3us);
                        # defer it so it never blocks the DVE queue head
                        def add():
                            nc.vector.tensor_tensor(
                                outT[:, m, fs: fs + 512], t1[:], t2[:],
                                op=AX.add)
                        pending_rope.append(add)

                    return [mm_part(0), mm_part(2), mm_part(4),
                            mm_part(6), part2]

                # ---- pair-0 projections (prologue; ScalarE evacuations).
                # The last chunk pair is deferred into the attention
                # filler queue -- the prologue would otherwise outrun the
                # HBM input loads. ----
                NPRO = max(1, NW - 1)     # prologue chunk count
                for ck in range(NPRO):
                    for it in qk_chunk_parts(wqT, qT, 0, ck,
                                             scalar_evac=True):
                        it()
                    for it in qk_chunk_parts(wkT, kT, 0, ck,
                                             scalar_evac=True):
                        it()
                    if ck:
                        flush_rope(2)
                flush_rope()

                # ---- deferred window-division epilogues ----
                pending_div = []

                def flush_div():
                    while pending_div:
                        pending_div.pop(0)()

                def epilogue(hpair, w, cps, direct=False):
                    """Evacuate ctx accumulators to SBUF (frees PSUM fast),
                    start the reciprocal/broadcast chain, defer the divide
                    and odd-head pack hop.  direct=True (kernel tail):
                    minimum-latency variant, nothing deferred.

                    Per head: denominator at accumulator row 64, ctx dims
                    at rows 0:64.  HW partition_broadcast only reads
                    partition 0, so the denominator row hops down via a
                    tiny DMA, is reciprocal'd there, and broadcast."""
                    ws = 512 * w
                    parts = []
                    for h2 in range(2):
                        cp = cps[h2]
                        if direct:
                            # tail: skip the evacuation hop -- reciprocal
                            # the whole accumulator straight from PSUM
                            # (offset-0 AP; the custom op is only unsafe
                            # at nonzero PSUM partition offsets)
                            src = cp
                            rec = small.tile([65, 512], F32, tag="rec")
                            nc.vector.reciprocal_approx_fast(
                                out=rec[0:65, :], in_=cp[0:65, :])
                            recr = rec[64:65, :]
                            rec0 = small.tile([1, 512], F32, tag="rec0")
                            (nc.sync if h2 == 0 else nc.scalar).dma_start(
                                rec0[:], rec[64:65, :])
                            recr = rec0[0:1, :]
                        else:
                            src = ctxsp.tile([P, 512], F32, tag="cs",
                                             name=f"cs_{hpair}_{w}_{h2}")
                            nc.vector.tensor_copy(src[0:65, :], cp[0:65, :])
                            rec0 = small.tile([1, 512], F32, tag="rec0")
                            (nc.sync if h2 == 0 else nc.gpsimd).dma_start(
                                rec0[:], src[64:65, :])
                            recr = small.tile([1, 512], F32, tag="recr")
                            nc.vector.reciprocal_approx_fast(
                                out=recr[0:1, :], in_=rec0[0:1, :])
                            recr = recr[0:1, :]
                        bcast = small.tile([64, 512], F32, tag="bc")
                        nc.gpsimd.partition_broadcast(bcast[:], recr)
                        parts.append((src, bcast))

                    def run(hpair=hpair, ws=ws, parts=parts,
                            direct=direct):
                        for h2, (src, bcast) in enumerate(parts):
                            nc.vector.tensor_tensor(
                                (ctx_pack if h2 == 0 else ctx_odd)[
                                    0:64, hpair, ws: ws + 512],
                                src[0:64, :], bcast[0:64, :], op=AX.mult)
                        # pack the odd-head rows into partitions 64:128;
                        # split across queues so the DMAs run in parallel
                        # (DMA-capable queues: SP, gpsimd, Activation)
                        nq = 3 if direct else 2
                        engs = [nc.sync, nc.gpsimd, nc.scalar]
                        bnds = [0, 32, 48, 64][:nq] + [64]
                        for i in range(nq):
                            lo, hi = bnds[i], bnds[i + 1]
                            engs[i].dma_start(
                                ctx_pack[64 + lo: 64 + hi,
                                         hpair, ws: ws + 512],
                                ctx_odd[lo: hi, hpair, ws: ws + 512])
                    if direct:
                        run()
                    else:
                        pending_div.append(run)

                # ---- output projection of one 512-token window, as a
                # list of small emission items (one [128,512] po tile
                # each) the attention loop pumps as PE filler.  cts
                # selects the ctx c-tiles to accumulate; addin names an
                # ostage tag holding a partial result to add on top ----
                def outproj_items(w, evac, cts=(0, 1), addin=None,
                                  otag="ot", store=True, pools=None):
                    ot = ostage.tile([P, 4, D], BF16, tag=otag,
                                     name=f"{otag}_{w}")
                    items = []
                    for ti in range(4):
                        t = 4 * w + ti
                        for nk in range(2):
                            def item(t=t, ti=ti, nk=nk):
                                if pools is None:
                                    po = qkpsum.tile([P, 512], F32,
                                                     tag="qk",
                                                     name=f"po_{t}_{nk}")
                                else:
                                    pool, tg = pools[(2 * ti + nk)
                                                     % len(pools)]
                                    po = pool.tile([P, 512], F32, tag=tg,
                                                   name=f"po_{t}_{nk}")
                                for i, ct in enumerate(cts):
                                    nc.tensor.matmul(
                                        po[:],
                                        lhsT=ctx_pack[:, ct,
                                                      P * t: P * t + P],
                                        rhs=woT[:, ct,
                                                512 * nk: 512 * nk + 512],
                                        start=(i == 0),
                                        stop=(i == len(cts) - 1),
                                    )
                                dst = ot[:, ti, 512 * nk: 512 * nk + 512]
                                e = evac[(2 * ti + nk) % len(evac)]
                                eng = {"s": nc.scalar, "v": nc.vector,
                                       "g": nc.gpsimd}[e]
                                if addin is None:
                                    if e == "s":
                                        eng.copy(dst, po[:])
                                    else:
                                        eng.tensor_copy(dst, po[:])
                                else:
                                    eng.tensor_tensor(
                                        dst, po[:],
                                        addin[:, ti,
                                              512 * nk: 512 * nk + 512],
                                        op=AX.add)
                            items.append(item)
                    if store == "split":
                        # one store per token tile, overlapping the
                        # remaining evacuations (kernel tail)
                        for ti in range(4):
                            items.insert(
                                2 * ti + 2 + ti,
                                lambda ti=ti: nc.sync.dma_start(
                                    out_d.rearrange(
                                        "(a p) d -> p a d", p=P)[
                                        :, 4 * w + ti: 4 * w + ti + 1, :],
                                    ot[:, ti: ti + 1, :]))
                    elif store:
                        items.append(lambda: nc.sync.dma_start(
                            out_d.rearrange("(a p) d -> p a d", p=P)[
                                :, 4 * w: 4 * w + 4, :], ot[:]))
                    return items, ot

                # ---- filler queue: small PE work items pumped one per
                # inner attention step, absorbing the per-j gap between
                # the PE's work (~0.73us) and ScalarE's exp (~0.87us) ----
                import collections as _c
                fillers = _c.deque()

                def pump(n=1):
                    for _ in range(n):
                        if not fillers:
                            return
                        fillers.popleft()()

                # ---- software-pipelined attention for one head pair ----
                def attention(hpair, boundary, boundary_j=1,
                              direct_last=False):
                    """boundary(w) is called at (w, j==boundary_j) -- it
                    refills the filler queue.  Deferred division
                    multiplies flush at (w, j==2), giving the broadcast
                    chain a head start so they never block the DVE queue
                    head (phase B's boundary runs at j==3, after the
                    flush, because its outproj consumes the divided ctx
                    of window w-1)."""
                    ch = hpair
                    prevs = _c.deque()   # 2-deep: (cps, w, j, jmax, d, at)

                    def emit_ctx(pv):
                        cps, pw, j, jmax, d, at = pv
                        for h2 in range(2):
                            h = 2 * hpair + h2
                            nc.tensor.matmul(
                                cps[h2][0:65, d: 512],
                                lhsT=vsb[:, j,
                                         (HD + 1) * h: (HD + 1) * h + HD + 1],
                                rhs=at[:, 512 * h2 + d: 512 * h2 + 512],
                                start=(j == 0), stop=(j == jmax - 1),
                            )
                        if j == jmax - 1:
                            epilogue(hpair, pw, cps,
                                     direct=(direct_last and pw == NW - 1))

                    for w in range(NW):
                        ws = 512 * w
                        jmax = (ws + 512) // 128
                        cps = {h2: cpsum.tile([P, 512], F32, tag="c",
                                              name=f"cp_{hpair}_{w}_{h2}")
                               for h2 in range(2)}
                        for j in range(jmax):
                            if j == min(4, jmax - 1):
                                flush_div()
                            if j == min(boundary_j, jmax - 1):
                                boundary(w)
                            start = max(ws, 128 * j)
                            d = start - ws
                            sc = spsum.tile([P, 1024], F32, tag="s",
                                            name=f"sc_{hpair}_{w}_{j}")
                            for h2 in range(2):
                                rh = 64 * h2
                                nc.tensor.matmul(
                                    sc[:, 512 * h2 + d: 512 * h2 + 512],
                                    lhsT=kT[rh: rh + 64, ch,
                                            128 * j: 128 * j + 128],
                                    rhs=qT[rh: rh + 64, ch, start: ws + 512],
                                    start=True, stop=True,
                                )
                            # trailing ctx from TWO j's back: the PE
                            # reaches it only after two newer scores, so
                            # it clears the in-order wait queue without
                            # ever stalling on exp at the queue head
                            if len(prevs) == 2:
                                emit_ctx(prevs.popleft())
                            at = attnp.tile([P, 1024], BF16, tag="attn",
                                            name=f"at_{hpair}_{w}_{j}")
                            nc.scalar.activation(
                                at[:].rearrange(
                                    "p (b n) -> p b n", b=2)[:, :, d: 512],
                                sc[:].rearrange(
                                    "p (b n) -> p b n", b=2)[:, :, d: 512],
                                mybir.ActivationFunctionType.Exp,
                                bias=0.0, scale=0.125,
                            )
                            if 128 * j >= ws:
                                # diagonal block: exp ran unmasked; zero
                                # the upper triangle of at AFTERWARD (off
                                # the scores->exp chain -- it only gates
                                # ctx, which has slack).  Phase B's DVE
                                # queue is the congested one; use gpsimd
                                # there.
                                atv = at[:].rearrange(
                                    "p (b n) -> p b n", b=2)[:, :, d: d + P]
                                # (last window's masks stay on DVE so the
                                # Pool queue is clear for the tail's
                                # broadcast chain)
                                meng = nc.gpsimd if (
                                    hpair and w < NW - 1) else nc.vector
                                meng.tensor_tensor(
                                    atv, atv,
                                    bmask[:, None, :].broadcast_to(
                                        [P, 2, P]),
                                    op=AX.mult)
                            prevs.append((cps, w, j, jmax, d, at))
                            pump(2 if len(fillers) > 16 else 1)
                    # drain remaining fillers (PE cover for the last
                    # exps) -- phase A keeps a few so phase B's
                    # filler-less first window has PE cover -- then the
                    # trailing ctxs + final epilogue
                    # (full-size only: the kept items are the last pair-1
                    # chunk, not consumed until phase B's last window)
                    keep = 10 if (not direct_last and NW >= 4) else 0
                    pump(max(0, len(fillers) - keep))
                    while prevs:
                        emit_ctx(prevs.popleft())

                # ---- phase A: pair-0 attention; fillers are the
                # deferred last pair-0 chunk, the deferred v-projection
                # tiles, and pair-1 projection chunks ----
                if NW > NPRO:
                    fillers.extend(qk_chunk_parts(wqT, qT, 0, NW - 1,
                                                  scalar_evac=False,
                                                  swap_sync=True))
                    fillers.extend(qk_chunk_parts(wkT, kT, 0, NW - 1,
                                                  scalar_evac=False,
                                                  swap_sync=True))
                    fillers.append(lambda: flush_rope(2))
                for t in range(TT // 2, TT):
                    fillers.extend(v_tile_parts(t, "v"))

                def boundary_a(w):
                    fillers.extend(qk_chunk_parts(wqT, qT, 1, w,
                                                  scalar_evac=False,
                                                  swap_sync=True))
                    fillers.extend(qk_chunk_parts(wkT, kT, 1, w,
                                                  scalar_evac=False,
                                                  swap_sync=True))
                    fillers.append(lambda: flush_rope(2))

                attention(0, boundary_a)

                # ---- phase B: pair-1 attention; output projections of
                # early windows as filler.  Window NW-2's projection is
                # HELD BACK: it runs in the tail as the PE work covering
                # the last window's division chain. ----
                def boundary_b(w):
                    flush_rope()
                    if 1 <= w <= NW - 2:
                        items, _ = outproj_items(w - 1, evac=("v", "v", "s", "v"))
                        fillers.extend(items)

                attention(1, boundary_b, boundary_j=5, direct_last=True)

                # ---- tail: window NW-2's projection covers the direct
                # division chain of window NW-1, then NW-1's projection
                # (ScalarE is exp-free by now and takes the evacuations)
                flush_div()
                if NW >= 2:
                    items, _ = outproj_items(
                        NW - 2, evac=("v", "s"),
                        pools=((qkpsum, "qk"), (spsum, "s")))
                    for it in items:
                        it()
                items, _ = outproj_items(
                    NW - 1, evac=("s", "v"),
                    pools=((qkpsum, "qk"), (spsum, "s"), (cpsum, "c")),
                    store="split")
                for it in items:
                    it()

    nc.compile()
    return nc


def make_in_maps(x, Wq, Wk, Wv, Wo, s=S):
    """Host-side shard prep: per-core input dict."""
    perm = head_perm()
    cosf, sinf = rope_tables(s)
    in_maps = []
    for c in range(NCORES):
        bi, hg = c // HG, c % HG
        heads = np.arange(HG * hg, HG * hg + HG)
        pcols = np.concatenate([h * HD + perm for h in heads])   # permuted q/k cols
        vcols = np.concatenate([h * HD + np.arange(HD) for h in heads])
        in_maps.append({
            "xT": np.ascontiguousarray(x[bi].T).astype(BF),
            "wqT": np.ascontiguousarray(Wq[pcols, :].T).astype(BF),
            "wkT": np.ascontiguousarray(Wk[pcols, :].T).astype(BF),
            "wvT": np.ascontiguousarray(Wv[vcols, :].T).astype(BF),
            "woT": np.ascontiguousarray(Wo[:, vcols].T).astype(BF),
            "cosf": cosf.astype(BF),
            "sinf": sinf.astype(BF),
        })
    return in_maps


_CACHE = {}


def _compiled(s=S):
    if s not in _CACHE:
        _CACHE[s] = build_kernel(s)
    return _CACHE[s]


def kernel(x, Wq, Wk, Wv, Wo, trace=False):
    x = np.asarray(x, dtype=np.float32)
    in_maps = make_in_maps(x, np.asarray(Wq), np.asarray(Wk),
                           np.asarray(Wv), np.asarray(Wo))
    nc = _compiled()
    res = run_bass_kernel_spmd(nc, in_maps, core_ids=list(range(NCORES)),
                               trace=trace)
    out = np.zeros((B, S, D), dtype=np.float32)
    for c in range(NCORES):
        out[c // HG] += res.results[c]["out"].astype(np.float32)
    if trace:
        return out, res
    return out


# revision 77
# speedup vs baseline: 447.0582x; 1.0185x over previous
"""Distributed causal self-attention (RoPE) kernel for 8 TRN2 NeuronCores.

Reference semantics (b=2, s=2048, d=1024, 16 heads, hd=64, fp32):
    q/k/v = x @ W{q,k,v}.T ; q,k = rope(q,k) ; causal softmax(q k^T/sqrt(hd)) @ v ; @ Wo.T

Sharding: core c -> batch (c // 4), head-group (c % 4) [4 heads = 256 dims].
Tensor-parallel column split of Wq/Wk/Wv, row split of Wo; the row-parallel
partial outputs are summed on the host (the unshard for this decomposition).
No device collectives.

Compute dtype: bf16 matmul operands and rope tables, fp32 PSUM
accumulation.  The head-dim basis is permuted per head to
[even dims | odd dims] (dot-product invariant, applied consistently to q
and k) so RoPE's rotate-half partner swap is a clean 32-partition-block
swap done by SBUF->SBUF DMA; the rope elementwise chain runs all-bf16 on
DVE after a single PSUM evacuation cast.  Softmax: scores are tiny
(|s| < 4) so no max subtraction; exp on ScalarE; the denominator comes
from a ones-column appended to V (row 64 of the ctx^T matmul
accumulator, exact in fp32), its reciprocal broadcast across partitions
via a tiny SP DMA hop + gpsimd partition_broadcast.

Schedule (PE is the bottleneck engine -- ~113us of matmul rows at 1
row/cycle bf16 -- everything else serves keeping it busy): single PSUM
pool scope for the whole kernel (spsum 4 banks + cpsum 2 + qkpsum 2,
no mid-kernel pool transitions).  The attention inner loop is
software-pipelined two deep: ctx(j) is emitted on the PE queue after
scores(j+2), so the in-order PE wait queue never stalls on exp; the
causal mask is applied multiplicatively to at AFTER the exp (bf16 DVE
2x / gpsimd) so the scores->exp chain has no cross-engine hop in it.
All non-attention PE work (the deferred half of the v projection, the
last pair-0 q/k chunk, all pair-1 q/k chunks, early output-projection
windows) is queued as small filler items pumped 1-2 per inner step,
absorbing the per-step gap between PE work (~0.73us) and ScalarE's exp
(~0.87us).  Window epilogues evacuate the ctx accumulator to SBUF with
one DVE copy (freeing the PSUM bank immediately); the denominator-row
hop / reciprocal / partition-broadcast chain runs off the critical
path with the divide and queue-split odd-head pack hop deferred several
inner steps.  Rope final adds are deferred so they never
head-of-line-block the DVE queue behind the partner-swap DMA.  The
tail runs window NW-2's output projection as PE cover for the last
window's direct division chain, spreads the last po tiles over three
PSUM pools, and splits the final store per token tile.
"""

import numpy as np
import ml_dtypes

import concourse.bass as bass
import concourse.mybir as mybir
import concourse.tile as tile
from concourse import bacc
from concourse.bass_utils import run_bass_kernel_spmd

P = 128
B, S, D = 2, 2048, 1024
NH, HD = 16, 64
NCORES = 8
HG = 4                 # heads per core
C = HG * HD            # 256 projected dims per core
THETA = 10000.0
F32 = mybir.dt.float32
BF16 = mybir.dt.bfloat16
BF = ml_dtypes.bfloat16

AX = mybir.AluOpType


def head_perm():
    """Per-head dim permutation: [0,2,...,62, 1,3,...,63]."""
    return np.arange(HD).reshape(HD // 2, 2).T.reshape(-1)


def rope_tables(s=S):
    """cosF/sinF [P, s] fp32 for the T-layout permuted basis.

    Row r (within a 128-row tile covering two heads): freq f = r % 32.
    sinF here is the PRE-SWAP table T with T[q] = S(partner(q)) * sin,
    i.e. +sin on the x1 half (r % 64 < 32), -sin on the x2 half, so that
    after the partner-block swap of t2pre = ps * T the rotate-half term
    lands with the right sign (see build_kernel).
    """
    inv = 1.0 / (THETA ** (np.arange(0, HD, 2, dtype=np.float64) / HD))  # [32]
    pos = np.arange(s, dtype=np.float64)
    r = np.arange(P)
    ang = pos[None, :] * inv[r % 32][:, None]          # [128, s]
    cosf = np.cos(ang).astype(np.float32)
    sign = np.where((r % 64) < 32, 1.0, -1.0)[:, None]
    sinf = (np.sin(ang) * sign).astype(np.float32)
    return cosf, sinf


def build_kernel(s=S, dbg=False, repeat=1):
    """Build the per-core Bass graph (same SPMD graph for all 8 cores)."""
    KT = D // P            # k-tiles over the model dim (8)
    CT = C // P            # partition tiles over this core's 256 dims (2)
    TT = s // P            # token tiles (16)
    NW = s // 512          # 512-wide q windows
    NEG = -1.0e30

    nc = bacc.Bacc("TRN2", target_bir_lowering=False, debug=False)

    xT_d = nc.dram_tensor("xT", [D, s], BF16, kind="ExternalInput").ap()
    wqT_d = nc.dram_tensor("wqT", [D, C], BF16, kind="ExternalInput").ap()
    wkT_d = nc.dram_tensor("wkT", [D, C], BF16, kind="ExternalInput").ap()
    wvT_d = nc.dram_tensor("wvT", [D, C], BF16, kind="ExternalInput").ap()
    woT_d = nc.dram_tensor("woT", [C, D], BF16, kind="ExternalInput").ap()
    cosf_d = nc.dram_tensor("cosf", [P, s], BF16, kind="ExternalInput").ap()
    sinf_d = nc.dram_tensor("sinf", [P, s], BF16, kind="ExternalInput").ap()
    out_d = nc.dram_tensor("out", [s, D], BF16, kind="ExternalOutput").ap()

    with tile.TileContext(nc) as tc:
      with (
          tc.tile_pool(name="persist", bufs=1) as persist,
          tc.tile_pool(name="small", bufs=3) as small,
      ):
        # ---- persistent SBUF staging ----
        wqT = persist.tile([P, KT, C], BF16, tag="wqT")
        wkT = persist.tile([P, KT, C], BF16, tag="wkT")
        wvT = persist.tile([P, KT, C], BF16, tag="wvT")
        woT = persist.tile([P, CT, D], BF16, tag="woT")
        cosf = persist.tile([P, s], BF16, tag="cosf")
        sinf = persist.tile([P, s], BF16, tag="sinf")
        qT = persist.tile([P, CT, s], BF16, tag="qT")
        kT = persist.tile([P, CT, s], BF16, tag="kT")
        # v with a ones column per head: [.., h*65+64] == 1.0
        vsb = persist.tile([P, TT, HG * (HD + 1)], BF16, tag="v")
        ctx_pack = persist.tile([P, CT, s], BF16, tag="ctxp")
        ctx_odd = persist.tile([P, CT, s], BF16, tag="ctxo")
        bmask = persist.tile([P, P], BF16, tag="bmask")

        for rep in range(repeat):
            with tc.tile_pool(name=f"xpool{rep}", bufs=1) as xpool, \
                 tc.tile_pool(name=f"ropet{rep}", bufs=3) as ropet, \
                 tc.tile_pool(name=f"attn{rep}", bufs=8) as attnp, \
                 tc.tile_pool(name=f"ctxs{rep}", bufs=4) as ctxsp, \
                 tc.tile_pool(name=f"ostage{rep}", bufs=2) as ostage, \
                 tc.tile_pool(name=f"spsum{rep}", bufs=2, space="PSUM") as spsum, \
                 tc.tile_pool(name=f"cpsum{rep}", bufs=2, space="PSUM") as cpsum, \
                 tc.tile_pool(name=f"qkpsum{rep}", bufs=2, space="PSUM") as qkpsum:
                xT = xpool.tile([P, KT, s], BF16, tag="xT", name="xT")
                xv = xT_d.rearrange("(a p) s -> p a s", p=P)
                # loads ordered by first use: v projection starts on the
                # first token tile, weights/tables land just before their
                # consumers, wo (tail-only) goes last
                def ldx(lo, hi):
                    if hi > lo:
                        nc.sync.dma_start(xT[:, :, lo:hi], xv[:, :, lo:hi])
                ldx(0, P)
                nc.sync.dma_start(wvT[:], wvT_d.rearrange("(a p) c -> p a c", p=P))
                ldx(P, max(s // 4, P))
                ldx(max(s // 4, P), s // 2)
                nc.sync.dma_start(wqT[:], wqT_d.rearrange("(a p) c -> p a c", p=P))
                nc.sync.dma_start(wkT[:], wkT_d.rearrange("(a p) c -> p a c", p=P))
                nc.sync.dma_start(sinf[:, 0: s // 2], sinf_d[:, 0: s // 2])
                nc.sync.dma_start(cosf[:, 0: s // 2], cosf_d[:, 0: s // 2])
                ldx(s // 2, 3 * s // 4)
                nc.sync.dma_start(sinf[:, s // 2: s], sinf_d[:, s // 2: s])
                nc.sync.dma_start(cosf[:, s // 2: s], cosf_d[:, s // 2: s])
                ldx(3 * s // 4, s)
                nc.sync.dma_start(woT[:], woT_d.rearrange("(a p) d -> p a d", p=P))

                # 0/1 causal mask [k-row, q-col]: 1 where qcol >= krow.
                # Applied multiplicatively to at AFTER the exp (bf16 DVE
                # 2x) so the scores->exp chain has no mask hop in it.
                nc.gpsimd.memset(bmask[:], 1.0)
                nc.gpsimd.affine_select(
                    out=bmask[:], in_=bmask[:],
                    compare_op=AX.is_ge, fill=0.0,
                    base=0, pattern=[[1, P]], channel_multiplier=-1,
                )
                # only the per-head ones columns need initializing; the v
                # projection fills the rest.
                nc.gpsimd.memset(
                    vsb[:].rearrange("p t (h e) -> p t h e", e=HD + 1)[
                        :, :, :, HD: HD + 1], 1.0)

                # ---- v projection (xT stationary -> natural layout).
                # Prologue tiles evacuate on ScalarE (idle until the
                # first exp); tiles deferred into the attention filler
                # queue evacuate on gpsimd (ScalarE is exp-bound there).
                def v_tile_parts(t, evac_eng):
                    state = {}

                    def half(k0):
                        def part():
                            if k0 == 0:
                                state["pv"] = qkpsum.tile(
                                    [P, 512], F32, tag="qk", name=f"pv_{t}")
                            pv = state["pv"]
                            for kt in range(k0, k0 + KT // 2):
                                nc.tensor.matmul(
                                    pv[:, 0:C],
                                    lhsT=xT[:, kt, P * t: P * t + P],
                                    rhs=wvT[:, kt, :],
                                    start=(kt == 0), stop=(kt == KT - 1),
                                )
                            if k0 == 0:
                                return
                            dst = vsb[:, t, :].rearrange(
                                "p (h e) -> p h e", e=HD + 1)[:, :, 0:HD]
                            src = pv[:, 0:C].rearrange(
                                "p (h e) -> p h e", e=HD)
                            if evac_eng == "s":
                                nc.scalar.copy(dst, src)
                            else:
                                nc.vector.tensor_copy(dst, src)
                        return part

                    return [half(0), half(KT // 2)]

                def v_tile(t, evac_eng):
                    for it in v_tile_parts(t, evac_eng):
                        it()

                for t in range(TT // 2):
                    v_tile(t, "s")

                # ---- q/k projection chunk, rope chain with deferred add ----
                pending_rope = []

                def flush_rope(n=None):
                    k = len(pending_rope) if n is None else min(n, len(pending_rope))
                    for _ in range(k):
                        pending_rope.pop(0)()

                def qk_chunk_parts(wT, outT, m, ck, scalar_evac,
                                   swap_sync=False):
                    """Two emission items (so the attention loop can pump
                    half a chunk per inner step as PE filler)."""
                    fs = 512 * ck
                    state = {}

                    def mm_part(k0):
                        def part():
                            if k0 == 0:
                                state["ps"] = qkpsum.tile(
                                    [P, 512], F32, tag="qk",
                                    name=f"ps_{m}_{ck}")
                            ps = state["ps"]
                            for kt in range(k0, k0 + 2):
                                nc.tensor.matmul(
                                    ps[:],
                                    lhsT=wT[:, kt, P * m: P * m + P],
                                    rhs=xT[:, kt, fs: fs + 512],
                                    start=(kt == 0), stop=(kt == KT - 1),
                                )
                        return part

                    def part2():
                        ps = state["ps"]
                        # evacuate the projection to bf16 SBUF once, then
                        # run the rope elementwise ops all-16-bit (DVE 2x)
                        psb = ropet.tile([P, 512], BF16, tag="psb")
                        if scalar_evac:
                            nc.scalar.copy(psb[:], ps[:])
                        else:
                            nc.vector.tensor_copy(psb[:], ps[:])
                        # t2pre[r] = psb[r] * sinF[partner(r)]; partner
                        # swap happens SBUF->SBUF by DMA (DMA cannot
                        # cross partitions on compute engines)
                        t2pre = ropet.tile([P, 512], BF16, tag="t2pre")
                        nc.vector.tensor_tensor(
                            t2pre[:], psb[:], sinf[:, fs: fs + 512],
                            op=AX.mult)
                        t2 = ropet.tile([P, 512], BF16, tag="t2")
                        for blk in range(4):
                            src = (blk ^ 1) * 32
                            eng = nc.sync if (swap_sync or blk % 2 == 0) \
                                else nc.gpsimd
                            eng.dma_start(
                                t2[32 * blk: 32 * blk + 32, :],
                                t2pre[src: src + 32, :])
                        t1 = ropet.tile([P, 512], BF16, tag="t1")
                        nc.vector.tensor_tensor(
                            t1[:], psb[:], cosf[:, fs: fs + 512],
                            op=AX.mult)

                        # the final add waits on the swap DMA (~1.3us);
                        # defer it so it never blocks the DVE queue head
                        def add():
                            nc.vector.tensor_tensor(
                                outT[:, m, fs: fs + 512], t1[:], t2[:],
                                op=AX.add)
                        pending_rope.append(add)

                    return [mm_part(0), mm_part(2), mm_part(4),
                            mm_part(6), part2]

                # ---- pair-0 projections (prologue; ScalarE evacuations).
                # The last chunk pair is deferred into the attention
                # filler queue -- the prologue would otherwise outrun the
                # HBM input loads. ----
                NPRO = max(1, NW - 2)     # prologue chunk count
                for ck in range(NPRO):
                    for it in qk_chunk_parts(wqT, qT, 0, ck,
                                             scalar_evac=True):
                        it()
                    for it in qk_chunk_parts(wkT, kT, 0, ck,
                                             scalar_evac=True):
                        it()
                    if ck:
                        flush_rope(2)
                flush_rope()

                # ---- deferred window-division epilogues ----
                pending_div = []

                def flush_div():
                    while pending_div:
                        pending_div.pop(0)()

                def epilogue(hpair, w, cps, direct=False):
                    """Evacuate ctx accumulators to SBUF (frees PSUM fast),
                    start the reciprocal/broadcast chain, defer the divide
                    and odd-head pack hop.  direct=True (kernel tail):
                    minimum-latency variant, nothing deferred.

                    Per head: denominator at accumulator row 64, ctx dims
                    at rows 0:64.  HW partition_broadcast only reads
                    partition 0, so the denominator row hops down via a
                    tiny DMA, is reciprocal'd there, and broadcast."""
                    ws = 512 * w
                    parts = []
                    for h2 in range(2):
                        cp = cps[h2]
                        if direct:
                            # tail: skip the evacuation hop -- reciprocal
                            # the whole accumulator straight from PSUM
                            # (offset-0 AP; the custom op is only unsafe
                            # at nonzero PSUM partition offsets)
                            src = cp
                            rec = small.tile([65, 512], F32, tag="rec")
                            nc.vector.reciprocal_approx_fast(
                                out=rec[0:65, :], in_=cp[0:65, :])
                            recr = rec[64:65, :]
                            rec0 = small.tile([1, 512], F32, tag="rec0")
                            (nc.sync if h2 == 0 else nc.scalar).dma_start(
                                rec0[:], rec[64:65, :])
                            recr = rec0[0:1, :]
                        else:
                            src = ctxsp.tile([P, 512], F32, tag="cs",
                                             name=f"cs_{hpair}_{w}_{h2}")
                            nc.vector.tensor_copy(src[0:65, :], cp[0:65, :])
                            rec0 = small.tile([1, 512], F32, tag="rec0")
                            (nc.sync if h2 == 0 else nc.gpsimd).dma_start(
                                rec0[:], src[64:65, :])
                            recr = small.tile([1, 512], F32, tag="recr")
                            nc.vector.reciprocal_approx_fast(
                                out=recr[0:1, :], in_=rec0[0:1, :])
                            recr = recr[0:1, :]
                        bcast = small.tile([64, 512], F32, tag="bc")
                        nc.gpsimd.partition_broadcast(bcast[:], recr)
                        parts.append((src, bcast))

                    def run(hpair=hpair, ws=ws, parts=parts,
                            direct=direct):
                        for h2, (src, bcast) in enumerate(parts):
                            nc.vector.tensor_tensor(
                                (ctx_pack if h2 == 0 else ctx_odd)[
                                    0:64, hpair, ws: ws + 512],
                                src[0:64, :], bcast[0:64, :], op=AX.mult)
                        # pack the odd-head rows into partitions 64:128;
                        # split across queues so the DMAs run in parallel
                        # (DMA-capable queues: SP, gpsimd, Activation)
                        nq = 3 if direct else 2
                        engs = [nc.sync, nc.gpsimd, nc.scalar]
                        bnds = [0, 32, 48, 64][:nq] + [64]
                        for i in range(nq):
                            lo, hi = bnds[i], bnds[i + 1]
                            engs[i].dma_start(
                                ctx_pack[64 + lo: 64 + hi,
                                         hpair, ws: ws + 512],
                                ctx_odd[lo: hi, hpair, ws: ws + 512])
                    if direct:
                        run()
                    else:
                        pending_div.append(run)

                # ---- output projection of one 512-token window, as a
                # list of small emission items (one [128,512] po tile
                # each) the attention loop pumps as PE filler.  cts
                # selects the ctx c-tiles to accumulate; addin names an
                # ostage tag holding a partial result to add on top ----
                def outproj_items(w, evac, cts=(0, 1), addin=None,
                                  otag="ot", store=True, pools=None):
                    ot = ostage.tile([P, 4, D], BF16, tag=otag,
                                     name=f"{otag}_{w}")
                    items = []
                    for ti in range(4):
                        t = 4 * w + ti
                        for nk in range(2):
                            def item(t=t, ti=ti, nk=nk):
                                if pools is None:
                                    po = qkpsum.tile([P, 512], F32,
                                                     tag="qk",
                                                     name=f"po_{t}_{nk}")
                                else:
                                    pool, tg = pools[(2 * ti + nk)
                                                     % len(pools)]
                                    po = pool.tile([P, 512], F32, tag=tg,
                                                   name=f"po_{t}_{nk}")
                                for i, ct in enumerate(cts):
                                    nc.tensor.matmul(
                                        po[:],
                                        lhsT=ctx_pack[:, ct,
                                                      P * t: P * t + P],
                                        rhs=woT[:, ct,
                                                512 * nk: 512 * nk + 512],
                                        start=(i == 0),
                                        stop=(i == len(cts) - 1),
                                    )
                                dst = ot[:, ti, 512 * nk: 512 * nk + 512]
                                e = evac[(2 * ti + nk) % len(evac)]
                                eng = {"s": nc.scalar, "v": nc.vector,
                                       "g": nc.gpsimd}[e]
                                if addin is None:
                                    if e == "s":
                                        eng.copy(dst, po[:])
                                    else:
                                        eng.tensor_copy(dst, po[:])
                                else:
                                    eng.tensor_tensor(
                                        dst, po[:],
                                        addin[:, ti,
                                              512 * nk: 512 * nk + 512],
                                        op=AX.add)
                            items.append(item)
                    if store == "split":
                        # one store per token tile, overlapping the
                        # remaining evacuations (kernel tail)
                        for ti in range(4):
                            items.insert(
                                2 * ti + 2 + ti,
                                lambda ti=ti: nc.sync.dma_start(
                                    out_d.rearrange(
                                        "(a p) d -> p a d", p=P)[
                                        :, 4 * w + ti: 4 * w + ti + 1, :],
                                    ot[:, ti: ti + 1, :]))
                    elif store:
                        items.append(lambda: nc.sync.dma_start(
                            out_d.rearrange("(a p) d -> p a d", p=P)[
                                :, 4 * w: 4 * w + 4, :], ot[:]))
                    return items, ot

                # ---- filler queue: small PE work items pumped one per
                # inner attention step, absorbing the per-j gap between
                # the PE's work (~0.73us) and ScalarE's exp (~0.87us) ----
                import collections as _c
                fillers = _c.deque()

                def pump(n=1):
                    for _ in range(n):
                        if not fillers:
                            return
                        fillers.popleft()()

                # ---- software-pipelined attention for one head pair ----
                def attention(hpair, boundary, boundary_j=1,
                              direct_last=False):
                    """boundary(w) is called at (w, j==boundary_j) -- it
                    refills the filler queue.  Deferred division
                    multiplies flush at (w, j==2), giving the broadcast
                    chain a head start so they never block the DVE queue
                    head (phase B's boundary runs at j==3, after the
                    flush, because its outproj consumes the divided ctx
                    of window w-1)."""
                    ch = hpair
                    prevs = _c.deque()   # 2-deep: (cps, w, j, jmax, d, at)

                    def emit_ctx(pv):
                        cps, pw, j, jmax, d, at = pv
                        for h2 in range(2):
                            h = 2 * hpair + h2
                            nc.tensor.matmul(
                                cps[h2][0:65, d: 512],
                                lhsT=vsb[:, j,
                                         (HD + 1) * h: (HD + 1) * h + HD + 1],
                                rhs=at[:, 512 * h2 + d: 512 * h2 + 512],
                                start=(j == 0), stop=(j == jmax - 1),
                            )
                        if j == jmax - 1:
                            epilogue(hpair, pw, cps,
                                     direct=(direct_last and pw == NW - 1))

                    for w in range(NW):
                        ws = 512 * w
                        jmax = (ws + 512) // 128
                        cps = {h2: cpsum.tile([P, 512], F32, tag="c",
                                              name=f"cp_{hpair}_{w}_{h2}")
                               for h2 in range(2)}
                        for j in range(jmax):
                            if j == min(4, jmax - 1):
                                flush_div()
                            if j == min(boundary_j, jmax - 1):
                                boundary(w)
                            start = max(ws, 128 * j)
                            d = start - ws
                            sc = spsum.tile([P, 1024], F32, tag="s",
                                            name=f"sc_{hpair}_{w}_{j}")
                            for h2 in range(2):
                                rh = 64 * h2
                                nc.tensor.matmul(
                                    sc[:, 512 * h2 + d: 512 * h2 + 512],
                                    lhsT=kT[rh: rh + 64, ch,
                                            128 * j: 128 * j + 128],
                                    rhs=qT[rh: rh + 64, ch, start: ws + 512],
                                    start=True, stop=True,
                                )
                            # trailing ctx from TWO j's back: the PE
                            # reaches it only after two newer scores, so
                            # it clears the in-order wait queue without
                            # ever stalling on exp at the queue head
                            if len(prevs) == 2:
                                emit_ctx(prevs.popleft())
                            at = attnp.tile([P, 1024], BF16, tag="attn",
                                            name=f"at_{hpair}_{w}_{j}")
                            nc.scalar.activation(
                                at[:].rearrange(
                                    "p (b n) -> p b n", b=2)[:, :, d: 512],
                                sc[:].rearrange(
                                    "p (b n) -> p b n", b=2)[:, :, d: 512],
                                mybir.ActivationFunctionType.Exp,
                                bias=0.0, scale=0.125,
                            )
                            if 128 * j >= ws:
                                # diagonal block: exp ran unmasked; zero
                                # the upper triangle of at AFTERWARD (off
                                # the scores->exp chain -- it only gates
                                # ctx, which has slack).  Phase B's DVE
                                # queue is the congested one; use gpsimd
                                # there.
                                atv = at[:].rearrange(
                                    "p (b n) -> p b n", b=2)[:, :, d: d + P]
                                # (last window's masks stay on DVE so the
                                # Pool queue is clear for the tail's
                                # broadcast chain)
                                meng = nc.gpsimd if (
                                    hpair and w < NW - 1) else nc.vector
                                meng.tensor_tensor(
                                    atv, atv,
                                    bmask[:, None, :].broadcast_to(
                                        [P, 2, P]),
                                    op=AX.mult)
                            prevs.append((cps, w, j, jmax, d, at))
                            pump(2 if len(fillers) > 16 else 1)
                    # drain remaining fillers (PE cover for the last
                    # exps) -- phase A keeps a few so phase B's
                    # filler-less first window has PE cover -- then the
                    # trailing ctxs + final epilogue
                    # (full-size only: the kept items are the last pair-1
                    # chunk, not consumed until phase B's last window)
                    keep = 10 if (not direct_last and NW >= 4) else 0
                    pump(max(0, len(fillers) - keep))
                    while prevs:
                        emit_ctx(prevs.popleft())

                # ---- phase A: pair-0 attention; fillers are the
                # deferred last pair-0 chunk, the deferred v-projection
                # tiles, and pair-1 projection chunks ----
                for ck in range(NPRO, NW):
                    fillers.extend(qk_chunk_parts(wqT, qT, 0, ck,
                                                  scalar_evac=False,
                                                  swap_sync=True))
                    fillers.extend(qk_chunk_parts(wkT, kT, 0, ck,
                                                  scalar_evac=False,
                                                  swap_sync=True))
                    fillers.append(lambda: flush_rope(2))
                for t in range(TT // 2, TT):
                    fillers.extend(v_tile_parts(t, "v"))

                def boundary_a(w):
                    fillers.extend(qk_chunk_parts(wqT, qT, 1, w,
                                                  scalar_evac=False,
                                                  swap_sync=True))
                    fillers.extend(qk_chunk_parts(wkT, kT, 1, w,
                                                  scalar_evac=False,
                                                  swap_sync=True))
                    fillers.append(lambda: flush_rope(2))

                attention(0, boundary_a)

                # ---- phase B: pair-1 attention; output projections of
                # early windows as filler.  Window NW-2's projection is
                # HELD BACK: it runs in the tail as the PE work covering
                # the last window's division chain. ----
                def boundary_b(w):
                    flush_rope()
                    if 1 <= w <= NW - 2:
                        items, _ = outproj_items(w - 1, evac=("v", "v", "s", "v"))
                        fillers.extend(items)

                attention(1, boundary_b, boundary_j=5, direct_last=True)

                # ---- tail: window NW-2's projection covers the direct
                # division chain of window NW-1, then NW-1's projection
                # (ScalarE is exp-free by now and takes the evacuations)
                flush_div()
                if NW >= 2:
                    items, _ = outproj_items(
                        NW - 2, evac=("v", "s"),
                        pools=((qkpsum, "qk"), (spsum, "s")))
                    for it in items:
                        it()
                items, _ = outproj_items(
                    NW - 1, evac=("s", "v"),
                    pools=((qkpsum, "qk"), (spsum, "s"), (cpsum, "c")),
                    store="split")
                for it in items:
                    it()

    nc.compile()
    return nc


def make_in_maps(x, Wq, Wk, Wv, Wo, s=S):
    """Host-side shard prep: per-core input dict."""
    perm = head_perm()
    cosf, sinf = rope_tables(s)
    in_maps = []
    for c in range(NCORES):
        bi, hg = c // HG, c % HG
        heads = np.arange(HG * hg, HG * hg + HG)
        pcols = np.concatenate([h * HD + perm for h in heads])   # permuted q/k cols
        vcols = np.concatenate([h * HD + np.arange(HD) for h in heads])
        in_maps.append({
            "xT": np.ascontiguousarray(x[bi].T).astype(BF),
            "wqT": np.ascontiguousarray(Wq[pcols, :].T).astype(BF),
            "wkT": np.ascontiguousarray(Wk[pcols, :].T).astype(BF),
            "wvT": np.ascontiguousarray(Wv[vcols, :].T).astype(BF),
            "woT": np.ascontiguousarray(Wo[:, vcols].T).astype(BF),
            "cosf": cosf.astype(BF),
            "sinf": sinf.astype(BF),
        })
    return in_maps


_CACHE = {}


def _compiled(s=S):
    if s not in _CACHE:
        _CACHE[s] = build_kernel(s)
    return _CACHE[s]


def kernel(x, Wq, Wk, Wv, Wo, trace=False):
    x = np.asarray(x, dtype=np.float32)
    in_maps = make_in_maps(x, np.asarray(Wq), np.asarray(Wk),
                           np.asarray(Wv), np.asarray(Wo))
    nc = _compiled()
    res = run_bass_kernel_spmd(nc, in_maps, core_ids=list(range(NCORES)),
                               trace=trace)
    out = np.zeros((B, S, D), dtype=np.float32)
    for c in range(NCORES):
        out[c // HG] += res.results[c]["out"].astype(np.float32)
    if trace:
        return out, res
    return out
